# revision 17
# baseline (speedup 1.0000x reference)
"""Trainium2 Bass kernel for nn_Blast: out = x @ (W0 + 1 bias^T) + bias
where W0 block (i_in, i_out) = Vt[i_in] @ diag(S[i_out, i_in]) @ U[i_out].

True 3-stage factorization in bf16 (16x fewer PE-streamed columns than the
merged Vt*S formulation):

  step1  y[(i,r), tok]  = Vt_blockdiag^T @ x        (32 MMs, K=128, N=256)
  step2  z[(o,r), tok]  = M2 @ y                    (8 MMs,  K=128, N=256)
  step3  out[tok, q]    = z_o^T @ U4_o              (16 MMs, K=128, N=512)

Layouts (per core, 256 tokens):
  y: 2 PSUM banks, row 16i+r; each step1 MM writes a 32-aligned pair-slot
     (i even -> weight cols 0-15, i odd -> 16-31, zero-padded) so M=32
     tile_position stays 32-granular.
  z: 2 PSUM banks, two [128,256] tok-tiles per bank; o -> tile o//4, rows
     32(o%4)+r; row 32(o%4)+16 carries (rowsum(x)+1) seeded by K=1 matmuls
     (e-vec x rs1) that also open each bank, so step3's U4 tables add
     (rowsum+1)*bias via their bias rows.
  U4: per z-tile g two [128,512] halves; half a maps o=4g+2a(+1) to disjoint
     512-col output segments with zero rows elsewhere, so one K=128 MM per
     (g, tok-half, half) emits a full [128 tok, 512 q] output tile. U4 is
     13% dense, so it ships compact (70KB) and expands via memset + 4 DMAs.
  out: 2 rotating [128,1024] PSUM pair-tiles (2 banks each); one engine copy
     per pair evacuates to SBUF bf16 (PSUM fp32 reads are the 1x DVE path,
     so copies pace step 3 across the two PSUM-capable engines).

All HBM traffic is bf16 (2 MB in, 2 MB out per core); host pre-transposes x
into [partition, chunk, token] layout so every DMA is a contiguous 2D slice
with 2 KB per partition line. x streams on two queues (sync: groups 0-3,
gpsimd: 4-7) and step 1 consumes groups in arrival order, running cold but
DMA-paced — the sustained stream itself lifts the PE clock gate (HAM) to
2.4 GHz for steps 2-3. Sharding: data-parallel over the 2048 tokens.
"""

import numpy as np
import ml_dtypes

IN_DIM = 4096
OUT_DIM = 4096
BLOCK = 256
RANK = 16
B_IN = 16
B_OUT = 16
N_CORES = 8
TOK = 2048
TPC = TOK // N_CORES          # 256 tokens per core
NCHUNK = IN_DIM // 128        # 32 K-chunks
XGRP = 4                      # chunks per x DMA group
GORDER = list(range(8))  # step-1 group processing = arrival order

BF16 = ml_dtypes.bfloat16

_CACHE = {}

# test.py toggles; harness never touches these
TRACE = False
TRACE_DIR = None
LAST_RESULTS = None


def build_program():
    import concourse.mybir as mybir
    from concourse import bacc
    from concourse.tile import TileContext

    f32 = mybir.dt.float32
    bf16 = mybir.dt.bfloat16

    nc = bacc.Bacc(trn_type="TRN2")
    xt_d = nc.dram_tensor("xt", (128, NCHUNK * TPC), bf16, kind="ExternalInput")
    w1_d = nc.dram_tensor("w1", (128, 1024), bf16, kind="ExternalInput")
    m2_d = nc.dram_tensor("m2", (128, 1024), bf16, kind="ExternalInput")
    ujc_d = nc.dram_tensor("ujc", (4, RANK + 1, 4, 256), bf16, kind="ExternalInput")
    rsv_d = nc.dram_tensor("rsv", (1, 384), bf16, kind="ExternalInput")
    out_d = nc.dram_tensor("out", (TPC, OUT_DIM), bf16, kind="ExternalOutput")

    with TileContext(nc) as tc:
        from contextlib import ExitStack

        with ExitStack() as ctx:
            consts = ctx.enter_context(tc.tile_pool(name="consts", bufs=1))
            xpool = ctx.enter_context(tc.tile_pool(name="xpool", bufs=1))
            ypool = ctx.enter_context(tc.tile_pool(name="ypool", bufs=1))
            zpool = ctx.enter_context(tc.tile_pool(name="zpool", bufs=1))
            opool = ctx.enter_context(tc.tile_pool(name="opool", bufs=8))
            ps = tc.alloc_tile_pool(name="ps", bufs=1, space="PSUM")
            ps_warm = tc.alloc_tile_pool(name="ps_warm", bufs=2, space="PSUM")

            # warmup weights come from a memset tile: no DMA dependency, so
            # the PE can start lifting the HAM clock gate at ~engine-start
            wtile = consts.tile([128, 512], bf16, name="wtile", tag="wtile")
            nc.vector.memset(wtile[:], 1.0)

            # ---- const loads: rsv+w1 on the scalar queue; m2 rides gpsimd
            # later so the early HBM window belongs to the x stream ----
            rssb = consts.tile([1, 384], bf16, name="rssb", tag="rssb")
            nc.scalar.dma_start(out=rssb[:], in_=rsv_d[:])
            w1sb = consts.tile([128, 1024], bf16, name="w1sb", tag="w1sb")
            nc.scalar.dma_start(out=w1sb[:], in_=w1_d[:])

            # U4 expansion: memset + 4 per-j placement DMAs, all on gpsimd
            # (idle until the output phase)
            u4sb = consts.tile([128, 4096], bf16, name="u4sb", tag="u4sb")
            nc.gpsimd.memset(u4sb[:], 0.0)
            m2sb = consts.tile([128, 1024], bf16, name="m2sb", tag="m2sb")
            nc.gpsimd.dma_start(out=m2sb[:], in_=m2_d[:])

            # ---- x stream: 4 chunks per transfer on the fast sync queue ----
            xg = [None] * (NCHUNK // XGRP)
            for b in range(8):
                xb = xpool.tile([128, XGRP * TPC], bf16, name=f"xb{b}", tag=f"xb{b}")
                nc.sync.dma_start(
                    out=xb[:], in_=xt_d[:, b * XGRP * TPC : (b + 1) * XGRP * TPC]
                )
                xg[b] = xb
            for j in range(4):
                nc.gpsimd.dma_start(
                    out=u4sb[32 * j : 32 * j + RANK + 1, :].rearrange(
                        "p (g q) -> p g q", g=4
                    )[
                        :,
                        :,
                        (j // 2) * 512 + (j % 2) * 256 : (j // 2) * 512
                        + (j % 2) * 256
                        + 256,
                    ],
                    in_=ujc_d[j],
                )

            # ---- PSUM: 2 y banks + 2 z banks (2 tok-tiles each) ----
            yps = [
                ps.tile([128, 512], f32, name=f"yps{c}", tag=f"yps{c}")
                for c in range(2)
            ]
            zps = [
                ps.tile([128, 512], f32, name=f"zpsb{b}", tag=f"zpsb{b}")
                for b in range(2)
            ]

            def zv(g):  # z tile for group g: bank g//2, half g%2
                return zps[g // 2][:, (g % 2) * TPC : (g % 2 + 1) * TPC]

            ysb = [
                ypool.tile([128, TPC], bf16, name=f"ysb{c}", tag=f"ysb{c}")
                for c in range(2)
            ]
            zsb = [
                zpool.tile([128, 2 * TPC], bf16, name=f"zsb{b}", tag=f"zsb{b}")
                for b in range(2)
            ]

            def s1_group(b):
                for kk in range(4):
                    k = 4 * b + kk
                    bank, co = k // 16, 32 * (b % 4)
                    nc.tensor.matmul(
                        yps[bank][co : co + 32, 0:TPC],
                        lhsT=w1sb[:, 32 * k : 32 * k + 32],
                        rhs=xg[b][:, kk * TPC : (kk + 1) * TPC],
                        start=(kk == 0),
                        stop=(kk == 3),
                        tile_position=(0, co),
                    )

            def s2_mm(c, g, stop):
                nc.tensor.matmul(
                    zv(g),
                    lhsT=m2sb[:, (4 * c + g) * 128 : (4 * c + g + 1) * 128],
                    rhs=ysb[c][:],
                    start=False,
                    stop=stop,
                    tile_position=(0, 0),
                )

            # ---- PE warmup on the memset tile (no DMA dependency) ----
            for w in range(10):
                warm = ps_warm.tile([128, 512], f32, name="warm", tag="warm")
                nc.tensor.matmul(
                    warm[:],
                    lhsT=wtile[:, 0:128],
                    rhs=wtile[:],
                    start=True,
                    stop=True,
                    tile_position=(0, 0),
                )

            # ---- step 1 in arrival order; rs seeds + step-2 c0 hidden ----
            for idx, b in enumerate(GORDER):
                s1_group(b)
                if idx == 0:
                    # open z banks with rowsum+1 in rows 32j+16 (start=True
                    # only on the first MM touching each bank)
                    for g in range(4):
                        nc.tensor.matmul(
                            zv(g),
                            lhsT=rssb[0:1, 256:384],
                            rhs=rssb[0:1, 0:TPC],
                            start=(g % 2 == 0),
                            stop=False,
                            tile_position=(0, 0),
                        )
                if idx == 3:  # y bank 0 (groups 0-3) complete
                    nc.scalar.copy(ysb[0][:], yps[0][:, 0:TPC])
                if idx == 5:
                    for g in range(4):
                        s2_mm(0, g, stop=False)
            nc.vector.tensor_copy(ysb[1][:], yps[1][:, 0:TPC])
            for g in (0, 1):
                s2_mm(1, g, stop=True)
            nc.scalar.copy(zsb[0][:], zps[0][:])
            for g in (2, 3):
                s2_mm(1, g, stop=True)
            nc.vector.tensor_copy(zsb[1][:], zps[1][:])

            # free the y/z/warm banks so step 3 can rotate 4 pair-tiles
            ps_warm.release()
            ps.release()
            ps_out = ctx.enter_context(
                tc.tile_pool(name="ps_out", bufs=4, space="PSUM")
            )

            def zslice(g, tt):
                base = (g % 2) * TPC + tt * 128
                return zsb[g // 2][:, base : base + 128]

            # ---- step 3: [128 tok, 2x512 q] pair tiles; one copy per pair ----
            for tt in range(2):
                for s2i in range(4):
                    osb_t = opool.tile([128, 1024], bf16, name="osb", tag="osb")
                    po = ps_out.tile([128, 1024], f32, name="po", tag="po")
                    for half in range(2):
                        s = 2 * s2i + half
                        g, a = s // 2, s % 2
                        nc.tensor.matmul(
                            po[:, half * 512 : (half + 1) * 512],
                            lhsT=zslice(g, tt),
                            rhs=u4sb[:, g * 1024 + a * 512 : g * 1024 + (a + 1) * 512],
                            start=True,
                            stop=True,
                            tile_position=(0, 0),
                        )
                    # split the evacuation across both PSUM-capable engines
                    nc.vector.tensor_copy(osb_t[:, 0:512], po[:, 0:512])
                    nc.scalar.copy(osb_t[:, 512:1024], po[:, 512:1024])
                    eng_dma = nc.gpsimd if tt == 0 else nc.sync
                    eng_dma.dma_start(
                        out=out_d[
                            tt * 128 : (tt + 1) * 128,
                            s2i * 1024 : (s2i + 1) * 1024,
                        ],
                        in_=osb_t[:],
                    )

    nc.compile()
    return nc


def prep_inputs(x, S, U, Vt, bias):
    """Host-side layout prep. Returns per-core input maps (all bf16)."""
    S = np.asarray(S, dtype=np.float32)
    U = np.asarray(U, dtype=np.float32)
    Vt = np.asarray(Vt, dtype=np.float32)
    bias = np.asarray(bias, dtype=np.float32)
    Xf = np.asarray(x, dtype=np.float32).reshape(TOK, IN_DIM)

    rowsum = Xf.sum(axis=1)
    xt_all = np.ascontiguousarray(Xf.T).astype(BF16)  # [4096, 2048]

    # step-1 weights: chunk k -> cols [32k, 32k+32), halves by i parity
    w1 = np.zeros((128, 1024), np.float32)
    for k in range(NCHUNK):
        i, h = k // 2, k % 2
        half = i % 2
        w1[:, 32 * k + 16 * half : 32 * k + 16 * half + 16] = Vt[
            i, 128 * h : 128 * h + 128, :
        ]

    # step-2 S-mixing blocks: (c,g) block maps y rows 16i'+r -> z rows 32j+r
    m2 = np.zeros((128, 1024), np.float32)
    r_idx = np.arange(RANK)
    for c in range(2):
        for g in range(4):
            blk = np.zeros((128, 128), np.float32)
            for ip in range(8):
                for j in range(4):
                    blk[16 * ip + r_idx, 32 * j + r_idx] = S[4 * g + j, 8 * c + ip, :]
            m2[:, (4 * c + g) * 128 : (4 * c + g + 1) * 128] = blk

    # compact step-3 tables: ujc[j, r, g, :] expands to u4 rows 32j+r at
    # cols g*1024 + (j//2)*512 + (j%2)*256; row RANK carries the bias
    ujc = np.zeros((4, RANK + 1, 4, 256), np.float32)
    for j in range(4):
        for g in range(4):
            o = 4 * g + j
            ujc[j, :RANK, g, :] = U[o]
            ujc[j, RANK, g, :] = bias[256 * o : 256 * o + 256]

    w1 = w1.astype(BF16)
    m2 = m2.astype(BF16)
    ujc = ujc.astype(BF16)

    in_maps = []
    for c in range(N_CORES):
        xt_c = np.ascontiguousarray(
            xt_all[:, TPC * c : TPC * (c + 1)]
            .reshape(NCHUNK, 128, TPC)
            .transpose(1, 0, 2)
            .reshape(128, NCHUNK * TPC)
        )
        rsv = np.zeros((1, 384), np.float32)
        rsv[0, :TPC] = rowsum[TPC * c : TPC * (c + 1)] + 1.0
        rsv[0, 256 + np.array([16, 48, 80, 112])] = 1.0
        in_maps.append(
            {
                "xt": xt_c,
                "w1": w1,
                "m2": m2,
                "ujc": ujc,
                "rsv": rsv.astype(BF16),
            }
        )
    return in_maps


def kernel(x, S, U, Vt, bias):
    global LAST_RESULTS
    from concourse.bass_utils import run_bass_kernel_spmd

    if "nc" not in _CACHE:
        _CACHE["nc"] = build_program()
    nc = _CACHE["nc"]

    in_maps = prep_inputs(x, S, U, Vt, bias)
    res = run_bass_kernel_spmd(
        nc, in_maps, list(range(N_CORES)), trace=TRACE, tmpdir=TRACE_DIR
    )
    LAST_RESULTS = res
    out = np.concatenate(
        [np.asarray(res.results[c]["out"], dtype=np.float32) for c in range(N_CORES)],
        axis=0,
    )
    return out.reshape(2, TOK // 2, OUT_DIM)


# revision 24
# speedup vs baseline: 1.0261x; 1.0261x over previous
"""Trainium2 Bass kernel for nn_Blast: out = x @ (W0 + 1 bias^T) + bias
where W0 block (i_in, i_out) = Vt[i_in] @ diag(S[i_out, i_in]) @ U[i_out].

True 3-stage factorization in bf16 (16x fewer PE-streamed columns than the
merged Vt*S formulation):

  step1  y[(i,r), tok]  = Vt_blockdiag^T @ x        (32 MMs, K=128, N=256)
  step2  z[(o,r), tok]  = M2 @ y                    (8 MMs,  K=128, N=256)
  step3  out[tok, q]    = z_o^T @ U4_o              (16 MMs, K=128, N=512)

Layouts (per core, 256 tokens):
  y: 2 PSUM banks, row 16i+r; each step1 MM writes a 32-aligned pair-slot
     (i even -> weight cols 0-15, i odd -> 16-31, zero-padded) so M=32
     tile_position stays 32-granular.
  z: 2 PSUM banks, two [128,256] tok-tiles per bank; o -> tile o//4, rows
     32(o%4)+r; row 32(o%4)+16 carries (rowsum(x)+1) seeded by K=1 matmuls
     (e-vec x rs1) that also open each bank, so step3's U4 tables add
     (rowsum+1)*bias via their bias rows.
  U4: per z-tile g two [128,512] halves; half a maps o=4g+2a(+1) to disjoint
     512-col output segments with zero rows elsewhere, so one K=128 MM per
     (g, tok-half, half) emits a full [128 tok, 512 q] output tile. U4 is
     13% dense, so it ships compact (70KB) and expands via memset + 4 DMAs.
  out: 2 rotating [128,1024] PSUM pair-tiles (2 banks each); one engine copy
     per pair evacuates to SBUF bf16 (PSUM fp32 reads are the 1x DVE path,
     so copies pace step 3 across the two PSUM-capable engines).

All HBM traffic is bf16 (2 MB in, 2 MB out per core); host pre-transposes x
into [partition, chunk, token] layout so every DMA is a contiguous 2D slice
with 2 KB per partition line. x streams on two queues (sync: groups 0-3,
gpsimd: 4-7) and step 1 consumes groups in arrival order, running cold but
DMA-paced — the sustained stream itself lifts the PE clock gate (HAM) to
2.4 GHz for steps 2-3. Sharding: data-parallel over the 2048 tokens.
"""

import numpy as np
import ml_dtypes

IN_DIM = 4096
OUT_DIM = 4096
BLOCK = 256
RANK = 16
B_IN = 16
B_OUT = 16
N_CORES = 8
TOK = 2048
TPC = TOK // N_CORES          # 256 tokens per core
NCHUNK = IN_DIM // 128        # 32 K-chunks
XGRP = 4                      # chunks per x DMA group
GORDER = list(range(8))  # step-1 group processing = arrival order

BF16 = ml_dtypes.bfloat16

_CACHE = {}

# test.py toggles; harness never touches these
TRACE = False
TRACE_DIR = None
LAST_RESULTS = None


def build_program():
    import concourse.mybir as mybir
    from concourse import bacc
    from concourse.tile import TileContext

    f32 = mybir.dt.float32
    bf16 = mybir.dt.bfloat16

    nc = bacc.Bacc(trn_type="TRN2")
    xt_d = nc.dram_tensor("xt", (128, NCHUNK * TPC), bf16, kind="ExternalInput")
    w1_d = nc.dram_tensor("w1", (128, 1024), bf16, kind="ExternalInput")
    m2_d = nc.dram_tensor("m2", (64, 2048), bf16, kind="ExternalInput")
    ujc_d = nc.dram_tensor("ujc", (4, RANK + 1, 4, 256), bf16, kind="ExternalInput")
    rsv_d = nc.dram_tensor("rsv", (1, 384), bf16, kind="ExternalInput")
    out_d = nc.dram_tensor("out", (TPC, OUT_DIM), bf16, kind="ExternalOutput")

    with TileContext(nc) as tc:
        from contextlib import ExitStack

        with ExitStack() as ctx:
            consts = ctx.enter_context(tc.tile_pool(name="consts", bufs=1))
            xpool = ctx.enter_context(tc.tile_pool(name="xpool", bufs=1))
            ypool = ctx.enter_context(tc.tile_pool(name="ypool", bufs=1))
            zpool = ctx.enter_context(tc.tile_pool(name="zpool", bufs=1))
            opool = ctx.enter_context(tc.tile_pool(name="opool", bufs=8))
            ps = tc.alloc_tile_pool(name="ps", bufs=1, space="PSUM")
            ps_warm = tc.alloc_tile_pool(name="ps_warm", bufs=2, space="PSUM")

            # warmup weights come from a memset tile: no DMA dependency, so
            # the PE can start lifting the HAM clock gate at ~engine-start
            wtile = consts.tile([128, 512], bf16, name="wtile", tag="wtile")
            nc.vector.memset(wtile[:], 1.0)

            # ---- const loads: rsv+w1 on the scalar queue; m2 rides gpsimd
            # later so the early HBM window belongs to the x stream ----
            rssb = consts.tile([1, 384], bf16, name="rssb", tag="rssb")
            nc.scalar.dma_start(out=rssb[:], in_=rsv_d[:])
            w1sb = consts.tile([128, 1024], bf16, name="w1sb", tag="w1sb")
            nc.scalar.dma_start(out=w1sb[:], in_=w1_d[:])

            # U4 expansion: memset + 4 per-j placement DMAs, all on gpsimd
            # (idle until the output phase); m2 rides behind the memset so it
            # stays out of the x stream's early HBM window
            u4sb = consts.tile([128, 4096], bf16, name="u4sb", tag="u4sb")
            nc.gpsimd.memset(u4sb[:], 0.0)
            m2sb = consts.tile([64, 2048], bf16, name="m2sb", tag="m2sb")
            nc.gpsimd.dma_start(out=m2sb[:], in_=m2_d[:])

            # ---- x stream: 4 chunks per transfer on the fast sync queue ----
            xg = [None] * (NCHUNK // XGRP)
            for b in range(8):
                xb = xpool.tile([128, XGRP * TPC], bf16, name=f"xb{b}", tag=f"xb{b}")
                nc.sync.dma_start(
                    out=xb[:], in_=xt_d[:, b * XGRP * TPC : (b + 1) * XGRP * TPC]
                )
                xg[b] = xb
            for j in range(4):
                nc.gpsimd.dma_start(
                    out=u4sb[32 * j : 32 * j + RANK + 1, :].rearrange(
                        "p (g q) -> p g q", g=4
                    )[
                        :,
                        :,
                        (j // 2) * 512 + (j % 2) * 256 : (j // 2) * 512
                        + (j % 2) * 256
                        + 256,
                    ],
                    in_=ujc_d[j],
                )

            # ---- PSUM: 4 quarter y banks + 2 z banks (2 tok-tiles each) ----
            yps = [
                ps.tile([128, 512], f32, name=f"yps{c}", tag=f"yps{c}")
                for c in range(4)
            ]
            zps = [
                ps.tile([128, 512], f32, name=f"zpsb{b}", tag=f"zpsb{b}")
                for b in range(2)
            ]

            def zv(g):  # z tile for group g: bank g//2, half g%2
                return zps[g // 2][:, (g % 2) * TPC : (g % 2 + 1) * TPC]

            ysb = [
                ypool.tile([64, TPC], bf16, name=f"ysb{c}", tag=f"ysb{c}")
                for c in range(4)
            ]
            zsb = [
                zpool.tile([128, 2 * TPC], bf16, name=f"zsb{b}", tag=f"zsb{b}")
                for b in range(2)
            ]

            def s1_group(b):
                for kk in range(4):
                    k = 4 * b + kk
                    bank, co = k // 8, 32 * (b % 2)
                    nc.tensor.matmul(
                        yps[bank][co : co + 32, 0:TPC],
                        lhsT=w1sb[:, 32 * k : 32 * k + 32],
                        rhs=xg[b][:, kk * TPC : (kk + 1) * TPC],
                        start=(kk == 0),
                        stop=(kk == 3),
                        tile_position=(0, co),
                    )

            def y_copy(c, eng):
                eng(ysb[c][:], yps[c][0:64, 0:TPC])

            def s2_mm(c, g, stop):
                nc.tensor.matmul(
                    zv(g),
                    lhsT=m2sb[:, (4 * c + g) * 128 : (4 * c + g + 1) * 128],
                    rhs=ysb[c][:],
                    start=False,
                    stop=stop,
                    tile_position=(0, 0),
                )

            def fill_mm():
                warm = ps_warm.tile([128, 512], f32, name="warm", tag="warm")
                nc.tensor.matmul(
                    warm[:],
                    lhsT=wtile[:, 0:128],
                    rhs=wtile[:],
                    start=True,
                    stop=True,
                    tile_position=(0, 0),
                )

            # ---- PE warmup on the memset tile (no DMA dependency) ----
            for w in range(10):
                warm = ps_warm.tile([128, 512], f32, name="warm", tag="warm")
                nc.tensor.matmul(
                    warm[:],
                    lhsT=wtile[:, 0:128],
                    rhs=wtile[:],
                    start=True,
                    stop=True,
                    tile_position=(0, 0),
                )

            # ---- step 1 in arrival order; y quarter-copies + step-2 chunks
            # run incrementally as groups land, so after the last x group
            # only one quarter of step 2 remains ----
            for idx, b in enumerate(GORDER):
                s1_group(b)
                if idx == 0:
                    # open z banks with rowsum+1 in rows 32j+16 (start=True
                    # only on the first MM touching each bank)
                    for g in range(4):
                        nc.tensor.matmul(
                            zv(g),
                            lhsT=rssb[0:1, 256:384],
                            rhs=rssb[0:1, 0:TPC],
                            start=(g % 2 == 0),
                            stop=False,
                            tile_position=(0, 0),
                        )
                if idx == 1:
                    y_copy(0, nc.scalar.copy)
                if idx == 3:
                    y_copy(1, nc.vector.tensor_copy)
                    fill_mm()
                if idx == 4:
                    for g in range(4):
                        s2_mm(0, g, stop=False)
                    fill_mm()
                if idx == 5:
                    y_copy(2, nc.scalar.copy)
                    for g in range(4):
                        s2_mm(1, g, stop=False)
                    fill_mm()
                if idx == 6:
                    for g in range(4):
                        s2_mm(2, g, stop=False)
                    fill_mm()
            y_copy(3, nc.vector.tensor_copy)
            for g in (0, 1):
                s2_mm(3, g, stop=True)
            nc.scalar.copy(zsb[0][:], zps[0][:])
            for g in (2, 3):
                s2_mm(3, g, stop=True)
            nc.vector.tensor_copy(zsb[1][:], zps[1][:])

            # free the y/z/warm banks so step 3 can rotate 4 pair-tiles
            ps_warm.release()
            ps.release()
            ps_out = ctx.enter_context(
                tc.tile_pool(name="ps_out", bufs=4, space="PSUM")
            )

            def zslice(g, tt):
                base = (g % 2) * TPC + tt * 128
                return zsb[g // 2][:, base : base + 128]

            # ---- step 3: [128 tok, 2x512 q] pair tiles; one copy per pair ----
            for tt in range(2):
                for s2i in range(4):
                    osb_t = opool.tile([128, 1024], bf16, name="osb", tag="osb")
                    po = ps_out.tile([128, 1024], f32, name="po", tag="po")
                    for half in range(2):
                        s = 2 * s2i + half
                        g, a = s // 2, s % 2
                        nc.tensor.matmul(
                            po[:, half * 512 : (half + 1) * 512],
                            lhsT=zslice(g, tt),
                            rhs=u4sb[:, g * 1024 + a * 512 : g * 1024 + (a + 1) * 512],
                            start=True,
                            stop=True,
                            tile_position=(0, 0),
                        )
                    # split the evacuation across both PSUM-capable engines
                    nc.vector.tensor_copy(osb_t[:, 0:512], po[:, 0:512])
                    nc.scalar.copy(osb_t[:, 512:1024], po[:, 512:1024])
                    eng_dma = nc.gpsimd if tt == 0 else nc.sync
                    eng_dma.dma_start(
                        out=out_d[
                            tt * 128 : (tt + 1) * 128,
                            s2i * 1024 : (s2i + 1) * 1024,
                        ],
                        in_=osb_t[:],
                    )

    nc.compile()
    return nc


def prep_inputs(x, S, U, Vt, bias):
    """Host-side layout prep. Returns per-core input maps (all bf16)."""
    S = np.asarray(S, dtype=np.float32)
    U = np.asarray(U, dtype=np.float32)
    Vt = np.asarray(Vt, dtype=np.float32)
    bias = np.asarray(bias, dtype=np.float32)
    Xf = np.asarray(x, dtype=np.float32).reshape(TOK, IN_DIM)

    rowsum = Xf.sum(axis=1)
    xt_all = np.ascontiguousarray(Xf.T).astype(BF16)  # [4096, 2048]

    # step-1 weights: chunk k -> cols [32k, 32k+32), halves by i parity
    w1 = np.zeros((128, 1024), np.float32)
    for k in range(NCHUNK):
        i, h = k // 2, k % 2
        half = i % 2
        w1[:, 32 * k + 16 * half : 32 * k + 16 * half + 16] = Vt[
            i, 128 * h : 128 * h + 128, :
        ]

    # step-2 S-mixing blocks: (c,g) block maps y-quarter rows 16ii+r
    # (i = 4c+ii) -> z rows 32j+r
    m2 = np.zeros((64, 2048), np.float32)
    r_idx = np.arange(RANK)
    for c in range(4):
        for g in range(4):
            blk = np.zeros((64, 128), np.float32)
            for ii in range(4):
                for j in range(4):
                    blk[16 * ii + r_idx, 32 * j + r_idx] = S[4 * g + j, 4 * c + ii, :]
            m2[:, (4 * c + g) * 128 : (4 * c + g + 1) * 128] = blk

    # compact step-3 tables: ujc[j, r, g, :] expands to u4 rows 32j+r at
    # cols g*1024 + (j//2)*512 + (j%2)*256; row RANK carries the bias
    ujc = np.zeros((4, RANK + 1, 4, 256), np.float32)
    for j in range(4):
        for g in range(4):
            o = 4 * g + j
            ujc[j, :RANK, g, :] = U[o]
            ujc[j, RANK, g, :] = bias[256 * o : 256 * o + 256]

    w1 = w1.astype(BF16)
    m2 = m2.astype(BF16)
    ujc = ujc.astype(BF16)

    in_maps = []
    for c in range(N_CORES):
        xt_c = np.ascontiguousarray(
            xt_all[:, TPC * c : TPC * (c + 1)]
            .reshape(NCHUNK, 128, TPC)
            .transpose(1, 0, 2)
            .reshape(128, NCHUNK * TPC)
        )
        rsv = np.zeros((1, 384), np.float32)
        rsv[0, :TPC] = rowsum[TPC * c : TPC * (c + 1)] + 1.0
        rsv[0, 256 + np.array([16, 48, 80, 112])] = 1.0
        in_maps.append(
            {
                "xt": xt_c,
                "w1": w1,
                "m2": m2,
                "ujc": ujc,
                "rsv": rsv.astype(BF16),
            }
        )
    return in_maps


def kernel(x, S, U, Vt, bias):
    global LAST_RESULTS
    from concourse.bass_utils import run_bass_kernel_spmd

    if "nc" not in _CACHE:
        _CACHE["nc"] = build_program()
    nc = _CACHE["nc"]

    in_maps = prep_inputs(x, S, U, Vt, bias)
    res = run_bass_kernel_spmd(
        nc, in_maps, list(range(N_CORES)), trace=TRACE, tmpdir=TRACE_DIR
    )
    LAST_RESULTS = res
    out = np.concatenate(
        [np.asarray(res.results[c]["out"], dtype=np.float32) for c in range(N_CORES)],
        axis=0,
    )
    return out.reshape(2, TOK // 2, OUT_DIM)
